# revision 14
# baseline (speedup 1.0000x reference)
"""DynamicSparseMoE grouped-GEMM kernel for 8 TRN2 NeuronCores.

out[t] = tokens[t] @ weight[exp_ids[t]]   (T=8192, E=8, D=2048 -> 2048)

Strategy (expert-parallel, host-side dispatch):
  - Host sorts tokens by expert; core e owns expert e's weight and its
    routed tokens, padded to a common capacity C (SPMD needs equal shapes).
  - Inputs are cast to fp16 on the host (PE runs fp16 at 1 cyc/row vs
    fp32's 4; PSUM accumulation stays fp32, measured rel-err ~3e-4).
  - Tokens are passed transposed ([D, C]): the stationary operand is a
    token block xT[d-block, 128 t] (one LDWEIGHTS per 4 matmuls), the
    moving operand is a weight slice w[d-block, 512 o], and PSUM gets
    out[t-block, o-slice] in the natural output orientation.
  - t-blocks are processed in pairs with the contraction (kb) loop
    outermost inside the pair: 8 PSUM banks hold 2x4 accumulation groups
    and the PE can start as soon as the first kb-block of x/w arrives
    instead of waiting for the whole 8 MB weight.
  - Everything (x, w) is SBUF-resident; out streams per t-block.
"""

import numpy as np

P = 128
D = 2048
E = 8
KB = D // P  # 16 contraction blocks
NOS = 4  # 4 moving slices of 512 over the 2048 output dim
NS = D // NOS  # 512

_cache = {}


def _ensure_imports():
    try:
        import concourse.bass  # noqa: F401
    except ImportError:
        import sys

        for p in ("/opt/trn_rl_repo", "/opt/pypackages"):
            if p not in sys.path:
                sys.path.append(p)


def _np_dt(compute_dt):
    if compute_dt == "float16":
        return np.float16
    import ml_dtypes

    return ml_dtypes.bfloat16


def _build(C, compute_dt="float16"):
    """Build + compile the per-core Bass program for capacity C."""
    _ensure_imports()
    import concourse.bacc as bacc
    import concourse.mybir as mybir
    import concourse.tile as tile

    cdt = getattr(mybir.dt, compute_dt)
    TB = C // P  # t-blocks

    nc = bacc.Bacc(None, target_bir_lowering=False, debug=False)
    n0 = 2 * P if C // P >= 2 else P
    xt0_d = nc.declare_dram_parameter("xt0", [P, KB * n0], cdt, isOutput=False)
    xt_d = nc.declare_dram_parameter("xt", [D, C], cdt, isOutput=False)
    w_d = nc.declare_dram_parameter("w", [D, D], cdt, isOutput=False)
    out_d = nc.declare_dram_parameter("out", [C, D], mybir.dt.float32, isOutput=True)

    xt_t = xt_d.rearrange("(k p) n -> p k n", p=P)  # [128, 16, C]
    w_t = w_d.rearrange("(k p) o -> p k o", p=P)  # [128, 16, 2048]

    pairs = [[tb for tb in (p0, p0 + 1) if tb < TB] for p0 in range(0, TB, 2)]

    with tile.TileContext(nc) as tc:
        with (
            tc.tile_pool(name="wp", bufs=1) as wp,
            tc.tile_pool(name="xp", bufs=1) as xp,
            tc.tile_pool(name="op", bufs=3) as op,
            tc.tile_pool(name="pp", bufs=8, space="PSUM") as pp,
        ):
            # Pair 0's stationary blocks come from a dedicated host-packed
            # contiguous tile loaded before the 8 MB weight stream; the
            # remaining x arrives after the weights, by which time pairs 1+
            # still lead the PE comfortably. The very first matmul only
            # needs xp0's first kb-blocks and w0's first o-slice, so those
            # land as small separate DMAs ahead of everything else.
            N0A = 2  # kb blocks in the first x chunk
            xp0a = xp.tile([P, N0A * n0], cdt, tag="xp0a")
            nc.sync.dma_start(xp0a[:], xt0_d[:, : N0A * n0])
            w0 = wp.tile([P, D], cdt, tag="w0")
            nc.sync.dma_start(w0[:, :NS], w_t[:, 0, :NS])
            nc.sync.dma_start(w0[:, NS:], w_t[:, 0, NS:])
            w1 = wp.tile([P, D], cdt, tag="w1")
            nc.sync.dma_start(w1[:], w_t[:, 1, :])
            xp0b = xp.tile([P, (KB - N0A) * n0], cdt, tag="xp0b")
            nc.sync.dma_start(xp0b[:], xt0_d[:, N0A * n0 :])

            def xp0(kb):
                if kb < N0A:
                    return xp0a[:, kb * n0 : (kb + 1) * n0]
                return xp0b[:, (kb - N0A) * n0 : (kb - N0A + 1) * n0]

            w_sb = [w0, w1]
            for kb in range(2, KB):
                w_k = wp.tile([P, D], cdt, tag=f"w{kb}")
                nc.sync.dma_start(w_k[:], w_t[:, kb, :])
                w_sb.append(w_k)
            # x fully SBUF-resident for normal capacities; for extreme expert
            # skew (C > 2944 would overflow SBUF) stream x per t-block pair.
            resident = C <= 2944
            if resident:
                x_sb = []
                for kb in range(KB):
                    xt_k = xp.tile([P, C], cdt, tag=f"x{kb}")
                    nc.sync.dma_start(xt_k[:], xt_t[:, kb, :])
                    x_sb.append(xt_k)
            else:
                x_pair = {}
                for pi in range(1, len(pairs)):
                    tbs = pairs[pi]
                    n = len(tbs) * P
                    t0 = tbs[0] * P
                    xpi = xp.tile(
                        [P, KB * n], cdt, tag="xpair", bufs=3, name=f"xpair{pi}"
                    )
                    nc.sync.dma_start(
                        xpi.rearrange("p (k n) -> p k n", k=KB),
                        xt_t[:, :, t0 : t0 + n],
                    )
                    x_pair[pi] = xpi

            def lhs(pi, kb, tb, ti, ntb):
                if pi == 0:
                    return xp0(kb)[:, ti * P : (ti + 1) * P]
                if resident:
                    return x_sb[kb][:, tb * P : (tb + 1) * P]
                return x_pair[pi][:, (kb * ntb + ti) * P : (kb * ntb + ti + 1) * P]

            # PE pre-warm: HAM keeps the PE clock-gated at 1.2 GHz until it
            # has seen ~3.4 us of sustained activity. Run dummy matmuls on
            # memset data during the initial DMA wait so the real matmuls
            # start at 2.4 GHz. They scribble on pair 0's first PSUM bank,
            # which the first real start=True matmul clears anyway.
            warm = xp.tile([P, 64], cdt, tag="warm")
            nc.gpsimd.memset(warm[:], 0.0)

            for pi, tbs in enumerate(pairs):
                last = pi == len(pairs) - 1
                ps = {
                    (ti, os): pp.tile(
                        [P, NS], mybir.dt.float32, tag="ps", name=f"ps_{pi}_{ti}_{os}"
                    )
                    for ti in range(len(tbs))
                    for os in range(NOS)
                }
                if pi == 0:
                    for _ in range(80):
                        nc.tensor.matmul(
                            ps[(0, 0)][:64, :64],
                            lhsT=warm[:, :64],
                            rhs=warm[:, :64],
                            start=True,
                            stop=True,
                        )
                for kb in range(KB):
                    for ti, tb in enumerate(tbs):
                        for os in range(NOS):
                            nc.tensor.matmul(
                                ps[(ti, os)][:],
                                lhsT=lhs(pi, kb, tb, ti, len(tbs)),
                                rhs=w_sb[kb][:, os * NS : (os + 1) * NS],
                                start=(kb == 0),
                                stop=(kb == KB - 1),
                            )
                for ti, tb in enumerate(tbs):
                    o_sb = op.tile([P, D], mybir.dt.float32, tag="o", name=f"o_{pi}_{ti}")
                    for os in range(NOS):
                        nc.vector.tensor_copy(
                            o_sb[:, os * NS : (os + 1) * NS], ps[(ti, os)][:]
                        )
                        if last:
                            # tail: stream each 512-slice out as soon as its
                            # copy lands instead of one 1 MB DMA at the end
                            nc.scalar.dma_start(
                                out_d[tb * P : (tb + 1) * P, os * NS : (os + 1) * NS],
                                o_sb[:, os * NS : (os + 1) * NS],
                            )
                    if not last:
                        nc.scalar.dma_start(out_d[tb * P : (tb + 1) * P, :], o_sb[:])
    nc.compile()
    return nc


def _get_nc(C, compute_dt):
    key = (C, compute_dt)
    if key not in _cache:
        _cache[key] = _build(C, compute_dt)
    return _cache[key]


def kernel(tokens, weight, exp_ids, _trace=False, _compute_dt="float16"):
    _ensure_imports()
    from concourse.bass_utils import run_bass_kernel_spmd

    tokens = np.asarray(tokens)
    weight = np.asarray(weight)
    exp_ids = np.asarray(exp_ids)
    T = tokens.shape[0]

    order = np.argsort(exp_ids, kind="stable")
    counts = np.bincount(exp_ids, minlength=E)
    C = max(int(-(-counts.max() // P) * P), NS)

    starts = np.zeros(E + 1, dtype=np.int64)
    np.cumsum(counts, out=starts[1:])

    npdt = _np_dt(_compute_dt)
    tokens_c = tokens.astype(npdt)
    weight_c = weight.astype(npdt)

    n0 = 2 * P if C // P >= 2 else P
    in_maps = []
    for e in range(E):
        idx = order[starts[e] : starts[e + 1]]
        xt = np.zeros((D, C), dtype=npdt)
        xt[:, : counts[e]] = tokens_c[idx].T
        # xt0: first-pair stationary blocks packed [p, kb*n0 + t] contiguously
        xt0 = np.ascontiguousarray(
            xt[:, :n0].reshape(KB, P, n0).transpose(1, 0, 2).reshape(P, KB * n0)
        )
        in_maps.append({"xt": xt, "xt0": xt0, "w": np.ascontiguousarray(weight_c[e])})

    nc = _get_nc(C, _compute_dt)
    res = run_bass_kernel_spmd(
        nc,
        in_maps,
        core_ids=list(range(E)),
        trace=_trace,
        trace_cores=list(range(E)) if _trace else None,
    )

    out = np.empty((T, D), dtype=np.float32)
    for e in range(E):
        idx = order[starts[e] : starts[e + 1]]
        out[idx] = res.results[e]["out"][: counts[e], :]
    if _trace:
        return out, res
    return out


# revision 15
# speedup vs baseline: 1.0243x; 1.0243x over previous
"""DynamicSparseMoE grouped-GEMM kernel for 8 TRN2 NeuronCores.

out[t] = tokens[t] @ weight[exp_ids[t]]   (T=8192, E=8, D=2048 -> 2048)

Strategy (expert-parallel, host-side dispatch):
  - Host sorts tokens by expert; core e owns expert e's weight and its
    routed tokens, padded to a common capacity C (SPMD needs equal shapes).
  - Inputs are cast to fp16 on the host (PE runs fp16 at 1 cyc/row vs
    fp32's 4; PSUM accumulation stays fp32, measured rel-err ~3e-4).
  - Tokens are passed transposed ([D, C]): the stationary operand is a
    token block xT[d-block, 128 t] (one LDWEIGHTS per 4 matmuls), the
    moving operand is a weight slice w[d-block, 512 o], and PSUM gets
    out[t-block, o-slice] in the natural output orientation.
  - t-blocks are processed in pairs with the contraction (kb) loop
    outermost inside the pair: 8 PSUM banks hold 2x4 accumulation groups
    and the PE can start as soon as the first kb-block of x/w arrives
    instead of waiting for the whole 8 MB weight.
  - Everything (x, w) is SBUF-resident; out streams per t-block.
"""

import numpy as np

P = 128
D = 2048
E = 8
KB = D // P  # 16 contraction blocks
NOS = 4  # 4 moving slices of 512 over the 2048 output dim
NS = D // NOS  # 512

_cache = {}


def _ensure_imports():
    try:
        import concourse.bass  # noqa: F401
    except ImportError:
        import sys

        for p in ("/opt/trn_rl_repo", "/opt/pypackages"):
            if p not in sys.path:
                sys.path.append(p)


def _np_dt(compute_dt):
    if compute_dt == "float16":
        return np.float16
    import ml_dtypes

    return ml_dtypes.bfloat16


def _build(C, compute_dt="float16"):
    """Build + compile the per-core Bass program for capacity C."""
    _ensure_imports()
    import concourse.bacc as bacc
    import concourse.mybir as mybir
    import concourse.tile as tile

    cdt = getattr(mybir.dt, compute_dt)
    TB = C // P  # t-blocks

    nc = bacc.Bacc(None, target_bir_lowering=False, debug=False)
    n0 = 2 * P if C // P >= 2 else P
    xt0_d = nc.declare_dram_parameter("xt0", [P, KB * n0], cdt, isOutput=False)
    xt_d = nc.declare_dram_parameter("xt", [D, C], cdt, isOutput=False)
    w_d = nc.declare_dram_parameter("w", [D, D], cdt, isOutput=False)
    out_d = nc.declare_dram_parameter("out", [C, D], cdt, isOutput=True)

    xt_t = xt_d.rearrange("(k p) n -> p k n", p=P)  # [128, 16, C]
    w_t = w_d.rearrange("(k p) o -> p k o", p=P)  # [128, 16, 2048]

    pairs = [[tb for tb in (p0, p0 + 1) if tb < TB] for p0 in range(0, TB, 2)]

    with tile.TileContext(nc) as tc:
        with (
            tc.tile_pool(name="wp", bufs=1) as wp,
            tc.tile_pool(name="xp", bufs=1) as xp,
            tc.tile_pool(name="op", bufs=3) as op,
            tc.tile_pool(name="pp", bufs=8, space="PSUM") as pp,
        ):
            # Pair 0's stationary blocks come from a dedicated host-packed
            # contiguous tile loaded before the 8 MB weight stream; the
            # remaining x arrives after the weights, by which time pairs 1+
            # still lead the PE comfortably. The very first matmul only
            # needs xp0's first kb-blocks and w0's first o-slice, so those
            # land as small separate DMAs ahead of everything else.
            N0A = 2  # kb blocks in the first x chunk
            xp0a = xp.tile([P, N0A * n0], cdt, tag="xp0a")
            nc.sync.dma_start(xp0a[:], xt0_d[:, : N0A * n0])
            w0 = wp.tile([P, D], cdt, tag="w0")
            nc.sync.dma_start(w0[:, :NS], w_t[:, 0, :NS])
            nc.sync.dma_start(w0[:, NS:], w_t[:, 0, NS:])
            w1 = wp.tile([P, D], cdt, tag="w1")
            nc.sync.dma_start(w1[:], w_t[:, 1, :])
            xp0b = xp.tile([P, (KB - N0A) * n0], cdt, tag="xp0b")
            nc.sync.dma_start(xp0b[:], xt0_d[:, N0A * n0 :])

            def xp0(kb):
                if kb < N0A:
                    return xp0a[:, kb * n0 : (kb + 1) * n0]
                return xp0b[:, (kb - N0A) * n0 : (kb - N0A + 1) * n0]

            w_sb = [w0, w1]
            for kb in range(2, KB):
                w_k = wp.tile([P, D], cdt, tag=f"w{kb}")
                nc.sync.dma_start(w_k[:], w_t[:, kb, :])
                w_sb.append(w_k)
            # x fully SBUF-resident for normal capacities; for extreme expert
            # skew (C > 2944 would overflow SBUF) stream x per t-block pair.
            resident = C <= 2944
            if resident:
                x_sb = []
                for kb in range(KB):
                    xt_k = xp.tile([P, C], cdt, tag=f"x{kb}")
                    nc.sync.dma_start(xt_k[:], xt_t[:, kb, :])
                    x_sb.append(xt_k)
            else:
                x_pair = {}
                for pi in range(1, len(pairs)):
                    tbs = pairs[pi]
                    n = len(tbs) * P
                    t0 = tbs[0] * P
                    xpi = xp.tile(
                        [P, KB * n], cdt, tag="xpair", bufs=3, name=f"xpair{pi}"
                    )
                    nc.sync.dma_start(
                        xpi.rearrange("p (k n) -> p k n", k=KB),
                        xt_t[:, :, t0 : t0 + n],
                    )
                    x_pair[pi] = xpi

            def lhs(pi, kb, tb, ti, ntb):
                if pi == 0:
                    return xp0(kb)[:, ti * P : (ti + 1) * P]
                if resident:
                    return x_sb[kb][:, tb * P : (tb + 1) * P]
                return x_pair[pi][:, (kb * ntb + ti) * P : (kb * ntb + ti + 1) * P]

            # PE pre-warm: HAM keeps the PE clock-gated at 1.2 GHz until it
            # has seen ~3.4 us of sustained activity. Run dummy matmuls on
            # memset data during the initial DMA wait so the real matmuls
            # start at 2.4 GHz. They scribble on pair 0's first PSUM bank,
            # which the first real start=True matmul clears anyway.
            warm = xp.tile([P, 64], cdt, tag="warm")
            nc.gpsimd.memset(warm[:], 0.0)

            for pi, tbs in enumerate(pairs):
                last = pi == len(pairs) - 1
                ps = {
                    (ti, os): pp.tile(
                        [P, NS], mybir.dt.float32, tag="ps", name=f"ps_{pi}_{ti}_{os}"
                    )
                    for ti in range(len(tbs))
                    for os in range(NOS)
                }
                if pi == 0:
                    for _ in range(80):
                        nc.tensor.matmul(
                            ps[(0, 0)][:64, :64],
                            lhsT=warm[:, :64],
                            rhs=warm[:, :64],
                            start=True,
                            stop=True,
                        )
                for kb in range(KB):
                    for ti, tb in enumerate(tbs):
                        for os in range(NOS):
                            nc.tensor.matmul(
                                ps[(ti, os)][:],
                                lhsT=lhs(pi, kb, tb, ti, len(tbs)),
                                rhs=w_sb[kb][:, os * NS : (os + 1) * NS],
                                start=(kb == 0),
                                stop=(kb == KB - 1),
                            )
                for ti, tb in enumerate(tbs):
                    o_sb = op.tile([P, D], cdt, tag="o", name=f"o_{pi}_{ti}")
                    for os in range(NOS):
                        nc.vector.tensor_copy(
                            o_sb[:, os * NS : (os + 1) * NS], ps[(ti, os)][:]
                        )
                        if last:
                            # tail: stream each 512-slice out as soon as its
                            # copy lands instead of one 1 MB DMA at the end
                            nc.scalar.dma_start(
                                out_d[tb * P : (tb + 1) * P, os * NS : (os + 1) * NS],
                                o_sb[:, os * NS : (os + 1) * NS],
                            )
                    if not last:
                        nc.scalar.dma_start(out_d[tb * P : (tb + 1) * P, :], o_sb[:])
    nc.compile()
    return nc


def _get_nc(C, compute_dt):
    key = (C, compute_dt)
    if key not in _cache:
        _cache[key] = _build(C, compute_dt)
    return _cache[key]


def kernel(tokens, weight, exp_ids, _trace=False, _compute_dt="float16"):
    _ensure_imports()
    from concourse.bass_utils import run_bass_kernel_spmd

    tokens = np.asarray(tokens)
    weight = np.asarray(weight)
    exp_ids = np.asarray(exp_ids)
    T = tokens.shape[0]

    order = np.argsort(exp_ids, kind="stable")
    counts = np.bincount(exp_ids, minlength=E)
    C = max(int(-(-counts.max() // P) * P), NS)

    starts = np.zeros(E + 1, dtype=np.int64)
    np.cumsum(counts, out=starts[1:])

    npdt = _np_dt(_compute_dt)
    tokens_c = tokens.astype(npdt)
    weight_c = weight.astype(npdt)

    n0 = 2 * P if C // P >= 2 else P
    in_maps = []
    for e in range(E):
        idx = order[starts[e] : starts[e + 1]]
        xt = np.zeros((D, C), dtype=npdt)
        xt[:, : counts[e]] = tokens_c[idx].T
        # xt0: first-pair stationary blocks packed [p, kb*n0 + t] contiguously
        xt0 = np.ascontiguousarray(
            xt[:, :n0].reshape(KB, P, n0).transpose(1, 0, 2).reshape(P, KB * n0)
        )
        in_maps.append({"xt": xt, "xt0": xt0, "w": np.ascontiguousarray(weight_c[e])})

    nc = _get_nc(C, _compute_dt)
    res = run_bass_kernel_spmd(
        nc,
        in_maps,
        core_ids=list(range(E)),
        trace=_trace,
        trace_cores=list(range(E)) if _trace else None,
    )

    out = np.empty((T, D), dtype=np.float32)
    for e in range(E):
        idx = order[starts[e] : starts[e + 1]]
        out[idx] = res.results[e]["out"][: counts[e], :].astype(np.float32)
    if _trace:
        return out, res
    return out


# revision 27
# speedup vs baseline: 1.0692x; 1.0437x over previous
"""DynamicSparseMoE grouped-GEMM kernel for 8 TRN2 NeuronCores.

out[t] = tokens[t] @ weight[exp_ids[t]]   (T=8192, E=8, D=2048 -> 2048)

Strategy (expert-parallel, host-side dispatch):
  - Host sorts tokens by expert; core e owns expert e's weight and its
    routed tokens, padded to a common capacity C (SPMD needs equal shapes).
  - Inputs are cast to fp16 on the host (PE runs fp16 at 1 cyc/row vs
    fp32's 4; PSUM accumulation stays fp32, measured rel-err ~3e-4).
  - Tokens are passed transposed ([D, C]): the stationary operand is a
    token block xT[d-block, 128 t] (one LDWEIGHTS per 4 matmuls), the
    moving operand is a weight slice w[d-block, 512 o], and PSUM gets
    out[t-block, o-slice] in the natural output orientation.
  - t-blocks are processed in pairs with the contraction (kb) loop
    outermost inside the pair: 8 PSUM banks hold 2x4 accumulation groups
    and the PE can start as soon as the first kb-block of x/w arrives
    instead of waiting for the whole 8 MB weight.
  - Everything (x, w) is SBUF-resident; out streams per t-block.
"""

import numpy as np

P = 128
D = 2048
E = 8
KB = D // P  # 16 contraction blocks
NOS = 4  # 4 moving slices of 512 over the 2048 output dim
NS = D // NOS  # 512

_cache = {}


def _ensure_imports():
    try:
        import concourse.bass  # noqa: F401
    except ImportError:
        import sys

        for p in ("/opt/trn_rl_repo", "/opt/pypackages"):
            if p not in sys.path:
                sys.path.append(p)


def _np_dt(compute_dt):
    if compute_dt == "float16":
        return np.float16
    import ml_dtypes

    return ml_dtypes.bfloat16


def _build(C, compute_dt="float16", last_m=128):
    """Build + compile the per-core Bass program for capacity C."""
    _ensure_imports()
    import concourse.bacc as bacc
    import concourse.mybir as mybir
    import concourse.tile as tile

    cdt = getattr(mybir.dt, compute_dt)
    TB = C // P  # t-blocks

    nc = bacc.Bacc(None, target_bir_lowering=False, debug=False)
    n0 = 2 * P if C // P >= 2 else P
    xt0_d = nc.declare_dram_parameter("xt0", [P, KB * n0], cdt, isOutput=False)
    xt_d = nc.declare_dram_parameter("xt", [D, C], cdt, isOutput=False)
    w_d = nc.declare_dram_parameter("w", [D, D], cdt, isOutput=False)
    out_d = nc.declare_dram_parameter("out", [C, D], cdt, isOutput=True)

    xt_t = xt_d.rearrange("(k p) n -> p k n", p=P)  # [128, 16, C]
    w_t = w_d.rearrange("(k p) o -> p k o", p=P)  # [128, 16, 2048]

    pairs = [[tb for tb in (p0, p0 + 1) if tb < TB] for p0 in range(0, TB, 2)]

    with tile.TileContext(nc) as tc:
        with (
            tc.tile_pool(name="wp", bufs=1) as wp,
            tc.tile_pool(name="xp", bufs=1) as xp,
            tc.tile_pool(name="op", bufs=3) as op,
            tc.tile_pool(name="pp", bufs=8, space="PSUM") as pp,
        ):
            # Pair 0's stationary blocks come from a dedicated host-packed
            # contiguous tile loaded before the 8 MB weight stream; the
            # remaining x arrives after the weights, by which time pairs 1+
            # still lead the PE comfortably. The very first matmul only
            # needs xp0's first kb-blocks and w0's first o-slice, so those
            # land as small separate DMAs ahead of everything else.
            # The weight streams as two half-width phases (os 0-1 then 2-3)
            # in separate tiles: pair 0 runs phase A PE-bound against the
            # half-rate A stream instead of dripping against full-width
            # per-kb arrivals, and phase B's data is resident by the time
            # A finishes.
            HD = D // 2  # 1024: columns per phase
            N0A = 2  # kb blocks in the first x chunk
            xp0a = xp.tile([P, N0A * n0], cdt, tag="xp0a")
            nc.sync.dma_start(xp0a[:], xt0_d[:, : N0A * n0])
            wA = [wp.tile([P, HD], cdt, tag=f"wA{kb}", name=f"wA{kb}") for kb in range(KB)]
            wB = [wp.tile([P, HD], cdt, tag=f"wB{kb}", name=f"wB{kb}") for kb in range(KB)]
            nc.sync.dma_start(wA[0][:, :NS], w_t[:, 0, :NS])
            nc.sync.dma_start(wA[0][:, NS:], w_t[:, 0, NS:HD])
            # xp0b rides the Scalar engine's HWDGE ring so it lands in
            # parallel with the uninterrupted wA cadence on the Sync ring.
            xp0b = xp.tile([P, (KB - N0A) * n0], cdt, tag="xp0b")
            nc.scalar.dma_start(xp0b[:], xt0_d[:, N0A * n0 :])
            for kb in range(1, KB):
                nc.sync.dma_start(wA[kb][:], w_t[:, kb, :HD])
            for kb in range(KB):
                nc.sync.dma_start(wB[kb][:], w_t[:, kb, HD:])

            def xp0(kb):
                if kb < N0A:
                    return xp0a[:, kb * n0 : (kb + 1) * n0]
                return xp0b[:, (kb - N0A) * n0 : (kb - N0A + 1) * n0]

            def w_slice(kb, os):
                if os < 2:
                    return wA[kb][:, os * NS : (os + 1) * NS]
                return wB[kb][:, (os - 2) * NS : (os - 1) * NS]
            # x fully SBUF-resident for normal capacities; for extreme expert
            # skew (C > 2944 would overflow SBUF) stream x per t-block pair.
            resident = C <= 2944
            if resident:
                x_sb = []
                for kb in range(KB):
                    xt_k = xp.tile([P, C], cdt, tag=f"x{kb}")
                    nc.sync.dma_start(xt_k[:], xt_t[:, kb, :])
                    x_sb.append(xt_k)
            else:
                x_pair = {}
                for pi in range(1, len(pairs)):
                    tbs = pairs[pi]
                    n = len(tbs) * P
                    t0 = tbs[0] * P
                    xpi = xp.tile(
                        [P, KB * n], cdt, tag="xpair", bufs=3, name=f"xpair{pi}"
                    )
                    nc.sync.dma_start(
                        xpi.rearrange("p (k n) -> p k n", k=KB),
                        xt_t[:, :, t0 : t0 + n],
                    )
                    x_pair[pi] = xpi

            def lhs(pi, kb, tb, ti, ntb):
                if pi == 0:
                    return xp0(kb)[:, ti * P : (ti + 1) * P]
                if resident:
                    return x_sb[kb][:, tb * P : (tb + 1) * P]
                return x_pair[pi][:, (kb * ntb + ti) * P : (kb * ntb + ti + 1) * P]

            # PE pre-warm: HAM keeps the PE clock-gated at 1.2 GHz until it
            # has seen ~3.4 us of sustained activity. Run dummy matmuls on
            # memset data during the initial DMA wait so the real matmuls
            # start at 2.4 GHz. They scribble on pair 0's first PSUM bank,
            # which the first real start=True matmul clears anyway.
            warm = xp.tile([P, 64], cdt, tag="warm")
            nc.gpsimd.memset(warm[:], 0.0)

            for pi, tbs in enumerate(pairs):
                last = pi == len(pairs) - 1
                ps = {
                    (ti, os): pp.tile(
                        [P, NS], mybir.dt.float32, tag="ps", name=f"ps_{pi}_{ti}_{os}"
                    )
                    for ti in range(len(tbs))
                    for os in range(NOS)
                }
                if pi == 0:
                    for _ in range(72):
                        nc.tensor.matmul(
                            ps[(0, 0)][:64, :64],
                            lhsT=warm[:, :64],
                            rhs=warm[:, :64],
                            start=True,
                            stop=True,
                        )
                if last and len(tbs) == 1 and last_m == 64:
                    # Packed final block: the real tokens fit in 64 stationary
                    # columns, so run os pairs (0,1) and (2,3) CONCURRENTLY in
                    # the PE array's two column-group halves (tile_position
                    # auto-derived from the output base partition). Odd os
                    # groups land on partitions 64-127 of their own PSUM bank
                    # (separate banks, so start=True bank-clears don't collide).
                    H = P // 2
                    tb = tbs[0]
                    for kb in range(KB):
                        for os in range(NOS):
                            dst = (
                                ps[(0, os)][:H, :]
                                if os % 2 == 0
                                else ps[(0, os)][H:, :]
                            )
                            nc.tensor.matmul(
                                dst,
                                lhsT=lhs(pi, kb, tb, 0, 1)[:, :H],
                                rhs=w_slice(kb, os),
                                start=(kb == 0),
                                stop=(kb == KB - 1),
                            )
                    o_sb = op.tile([P, D], cdt, tag="o", name=f"oq_{pi}")
                    for os in range(NOS):
                        rows = slice(0, H) if os % 2 == 0 else slice(H, P)
                        nc.vector.tensor_copy(
                            o_sb[rows, os * NS : (os + 1) * NS],
                            ps[(0, os)][rows, :],
                        )
                        nc.scalar.dma_start(
                            out_d[tb * P : tb * P + H, os * NS : (os + 1) * NS],
                            o_sb[rows, os * NS : (os + 1) * NS],
                        )
                    continue
                # pair 0 follows the two-phase weight stream (os 0-1 while
                # the A halves land, then os 2-3); later pairs interleave
                # all four os per kb for 4-matmul LDWEIGHTS amortization.
                os_phases = [(0, 1), (2, 3)] if pi == 0 else [(0, 1, 2, 3)]
                for phase in os_phases:
                    for kb in range(KB):
                        for ti, tb in enumerate(tbs):
                            for os in phase:
                                nc.tensor.matmul(
                                    ps[(ti, os)][:],
                                    lhsT=lhs(pi, kb, tb, ti, len(tbs)),
                                    rhs=w_slice(kb, os),
                                    start=(kb == 0),
                                    stop=(kb == KB - 1),
                                )
                for ti, tb in enumerate(tbs):
                    o_sb = op.tile([P, D], cdt, tag="o", name=f"o_{pi}_{ti}")
                    for os in range(NOS):
                        nc.vector.tensor_copy(
                            o_sb[:, os * NS : (os + 1) * NS], ps[(ti, os)][:]
                        )
                        if last:
                            # tail: stream each 512-slice out as soon as its
                            # copy lands instead of one 1 MB DMA at the end
                            nc.scalar.dma_start(
                                out_d[tb * P : (tb + 1) * P, os * NS : (os + 1) * NS],
                                o_sb[:, os * NS : (os + 1) * NS],
                            )
                    if not last:
                        nc.scalar.dma_start(out_d[tb * P : (tb + 1) * P, :], o_sb[:])
    nc.compile()
    return nc


def _get_nc(C, compute_dt, last_m):
    key = (C, compute_dt, last_m)
    if key not in _cache:
        _cache[key] = _build(C, compute_dt, last_m)
    return _cache[key]


def kernel(tokens, weight, exp_ids, _trace=False, _compute_dt="float16"):
    _ensure_imports()
    from concourse.bass_utils import run_bass_kernel_spmd

    tokens = np.asarray(tokens)
    weight = np.asarray(weight)
    exp_ids = np.asarray(exp_ids)
    T = tokens.shape[0]

    order = np.argsort(exp_ids, kind="stable")
    counts = np.bincount(exp_ids, minlength=E)
    C = max(int(-(-counts.max() // P) * P), NS)

    starts = np.zeros(E + 1, dtype=np.int64)
    np.cumsum(counts, out=starts[1:])

    # Packed final block is valid when the last 128-block holds <= 64 real
    # tokens on every core and the block count is odd (lone final block).
    TB = C // P
    rest = int(counts.max()) - (TB - 1) * P
    last_m = 64 if (TB >= 3 and TB % 2 == 1 and rest <= 64) else 128

    npdt = _np_dt(_compute_dt)
    tokens_c = tokens.astype(npdt)
    weight_c = weight.astype(npdt)

    n0 = 2 * P if C // P >= 2 else P
    in_maps = []
    for e in range(E):
        idx = order[starts[e] : starts[e + 1]]
        xt = np.zeros((D, C), dtype=npdt)
        xt[:, : counts[e]] = tokens_c[idx].T
        # xt0: first-pair stationary blocks packed [p, kb*n0 + t] contiguously
        xt0 = np.ascontiguousarray(
            xt[:, :n0].reshape(KB, P, n0).transpose(1, 0, 2).reshape(P, KB * n0)
        )
        in_maps.append({"xt": xt, "xt0": xt0, "w": np.ascontiguousarray(weight_c[e])})

    nc = _get_nc(C, _compute_dt, last_m)
    res = run_bass_kernel_spmd(
        nc,
        in_maps,
        core_ids=list(range(E)),
        trace=_trace,
        trace_cores=list(range(E)) if _trace else None,
    )

    out = np.empty((T, D), dtype=np.float32)
    for e in range(E):
        idx = order[starts[e] : starts[e + 1]]
        out[idx] = res.results[e]["out"][: counts[e], :].astype(np.float32)
    if _trace:
        return out, res
    return out


# revision 33
# speedup vs baseline: 1.0928x; 1.0221x over previous
"""DynamicSparseMoE grouped-GEMM kernel for 8 TRN2 NeuronCores.

out[t] = tokens[t] @ weight[exp_ids[t]]   (T=8192, E=8, D=2048 -> 2048)

Strategy (expert-parallel, host-side dispatch):
  - Host sorts tokens by expert; core e owns expert e's weight and its
    routed tokens, padded to a common capacity C (SPMD needs equal shapes).
  - Inputs are cast to fp16 on the host (PE runs fp16 at 1 cyc/row vs
    fp32's 4; PSUM accumulation stays fp32; rel-err ~3.6e-4 end to end).
  - Tokens are passed transposed ([D, C]): the stationary operand is a
    token block xT[d-block, 128 t] (one LDWEIGHTS per 4 matmuls), the
    moving operand is a weight slice w[d-block, 512 o], and PSUM gets
    out[t-block, o-slice] in the natural output orientation.
  - t-blocks run in pairs, contraction (kb) loop outermost inside the
    pair: 8 PSUM banks hold 2x4 accumulation groups. Pair 0 rides the
    startup DMA stream: the weight arrives as two half-width phases
    (wA = o 0-1023 per kb, then wB) so phase A is PE-bound against the
    half-rate stream; pair 0's stationary tokens come from a dedicated
    host-packed tile, its bulk on the Scalar DMA ring in parallel with
    the Sync ring's weight cadence. ~96 warm-up matmuls on a memset
    tile hold the HAM clock-gate open until real data lands.
  - The final partial block (<=64 real tokens) runs its four o-slices
    as two CONCURRENT column-group-packed matmul pairs (tile_position
    via PSUM base-partition 0/64, separate banks so start=True bank
    clears don't collide), halving its cost.
  - x and w are SBUF-resident (pair-streamed x fallback for extreme
    skew); output streams per t-block as fp16 on the Scalar ring and
    the host casts back to f32 and unpermutes.
"""

import os

import numpy as np

# A previously wedged NeuronCore (NRT_EXEC_UNIT_UNRECOVERABLE) recovers on
# the next init when core reset is requested; must be set before NRT init.
os.environ.setdefault("NEURON_RT_RESET_CORES", "1")

P = 128
D = 2048
E = 8
KB = D // P  # 16 contraction blocks
NOS = 4  # 4 moving slices of 512 over the 2048 output dim
NS = D // NOS  # 512

_cache = {}


def _ensure_imports():
    try:
        import concourse.bass  # noqa: F401
    except ImportError:
        import sys

        for p in ("/opt/trn_rl_repo", "/opt/pypackages"):
            if p not in sys.path:
                sys.path.append(p)


def _np_dt(compute_dt):
    if compute_dt == "float16":
        return np.float16
    import ml_dtypes

    return ml_dtypes.bfloat16


def _build(C, compute_dt="float16", last_m=128):
    """Build + compile the per-core Bass program for capacity C."""
    _ensure_imports()
    import concourse.bacc as bacc
    import concourse.mybir as mybir
    import concourse.tile as tile

    cdt = getattr(mybir.dt, compute_dt)
    TB = C // P  # t-blocks

    nc = bacc.Bacc(None, target_bir_lowering=False, debug=False)
    n0 = 2 * P if C // P >= 2 else P
    xt0_d = nc.declare_dram_parameter("xt0", [P, KB * n0], cdt, isOutput=False)
    xt_d = nc.declare_dram_parameter("xt", [D, C], cdt, isOutput=False)
    w_d = nc.declare_dram_parameter("w", [D, D], cdt, isOutput=False)
    out_d = nc.declare_dram_parameter("out", [C, D], cdt, isOutput=True)

    xt_t = xt_d.rearrange("(k p) n -> p k n", p=P)  # [128, 16, C]
    w_t = w_d.rearrange("(k p) o -> p k o", p=P)  # [128, 16, 2048]

    pairs = [[tb for tb in (p0, p0 + 1) if tb < TB] for p0 in range(0, TB, 2)]

    with tile.TileContext(nc) as tc:
        with (
            tc.tile_pool(name="wp", bufs=1) as wp,
            tc.tile_pool(name="xp", bufs=1) as xp,
            tc.tile_pool(name="op", bufs=3) as op,
            tc.tile_pool(name="pp", bufs=8, space="PSUM") as pp,
        ):
            # Pair 0's stationary blocks come from a dedicated host-packed
            # contiguous tile loaded before the 8 MB weight stream; the
            # remaining x arrives after the weights, by which time pairs 1+
            # still lead the PE comfortably. The very first matmul only
            # needs xp0's first kb-blocks and w0's first o-slice, so those
            # land as small separate DMAs ahead of everything else.
            # The weight streams as two half-width phases (os 0-1 then 2-3)
            # in separate tiles: pair 0 runs phase A PE-bound against the
            # half-rate A stream instead of dripping against full-width
            # per-kb arrivals, and phase B's data is resident by the time
            # A finishes.
            HD = D // 2  # 1024: columns per phase
            N0A = 2  # kb blocks in the first x chunk
            xp0a = xp.tile([P, N0A * n0], cdt, tag="xp0a")
            nc.sync.dma_start(xp0a[:], xt0_d[:, : N0A * n0])
            wA = [wp.tile([P, HD], cdt, tag=f"wA{kb}", name=f"wA{kb}") for kb in range(KB)]
            wB = [wp.tile([P, HD], cdt, tag=f"wB{kb}", name=f"wB{kb}") for kb in range(KB)]
            nc.sync.dma_start(wA[0][:, :NS], w_t[:, 0, :NS])
            nc.sync.dma_start(wA[0][:, NS:], w_t[:, 0, NS:HD])
            # xp0b rides the Scalar engine's HWDGE ring so it lands in
            # parallel with the uninterrupted wA cadence on the Sync ring.
            xp0b = xp.tile([P, (KB - N0A) * n0], cdt, tag="xp0b")
            nc.scalar.dma_start(xp0b[:], xt0_d[:, N0A * n0 :])
            for kb in range(1, KB):
                nc.sync.dma_start(wA[kb][:], w_t[:, kb, :HD])
            for kb in range(KB):
                nc.sync.dma_start(wB[kb][:], w_t[:, kb, HD:])

            def xp0(kb):
                if kb < N0A:
                    return xp0a[:, kb * n0 : (kb + 1) * n0]
                return xp0b[:, (kb - N0A) * n0 : (kb - N0A + 1) * n0]

            def w_slice(kb, os):
                if os < 2:
                    return wA[kb][:, os * NS : (os + 1) * NS]
                return wB[kb][:, (os - 2) * NS : (os - 1) * NS]
            # x fully SBUF-resident for normal capacities; for extreme expert
            # skew (C > 2944 would overflow SBUF) stream x per t-block pair.
            resident = C <= 2944
            if resident:
                x_sb = []
                for kb in range(KB):
                    xt_k = xp.tile([P, C], cdt, tag=f"x{kb}")
                    nc.sync.dma_start(xt_k[:], xt_t[:, kb, :])
                    x_sb.append(xt_k)
            else:
                x_pair = {}
                for pi in range(1, len(pairs)):
                    tbs = pairs[pi]
                    n = len(tbs) * P
                    t0 = tbs[0] * P
                    xpi = xp.tile(
                        [P, KB * n], cdt, tag="xpair", bufs=3, name=f"xpair{pi}"
                    )
                    nc.sync.dma_start(
                        xpi.rearrange("p (k n) -> p k n", k=KB),
                        xt_t[:, :, t0 : t0 + n],
                    )
                    x_pair[pi] = xpi

            def lhs(pi, kb, tb, ti, ntb):
                if pi == 0:
                    return xp0(kb)[:, ti * P : (ti + 1) * P]
                if resident:
                    return x_sb[kb][:, tb * P : (tb + 1) * P]
                return x_pair[pi][:, (kb * ntb + ti) * P : (kb * ntb + ti + 1) * P]

            # PE pre-warm: HAM keeps the PE clock-gated at 1.2 GHz until it
            # has seen ~3.4 us of sustained activity. Run dummy matmuls on
            # memset data during the initial DMA wait so the real matmuls
            # start at 2.4 GHz. They scribble on pair 0's first PSUM bank,
            # which the first real start=True matmul clears anyway.
            warm = xp.tile([P, 64], cdt, tag="warm")
            nc.vector.memset(warm[:], 0.0)

            for pi, tbs in enumerate(pairs):
                last = pi == len(pairs) - 1
                ps = {
                    (ti, os): pp.tile(
                        [P, NS], mybir.dt.float32, tag="ps", name=f"ps_{pi}_{ti}_{os}"
                    )
                    for ti in range(len(tbs))
                    for os in range(NOS)
                }
                if pi == 0:
                    for _ in range(72):
                        nc.tensor.matmul(
                            ps[(0, 0)][:64, :64],
                            lhsT=warm[:, :64],
                            rhs=warm[:, :64],
                            start=True,
                            stop=True,
                        )
                if last and len(tbs) == 1 and last_m == 64:
                    # Packed final block: the real tokens fit in 64 stationary
                    # columns, so run os pairs (0,1) and (2,3) CONCURRENTLY in
                    # the PE array's two column-group halves (tile_position
                    # auto-derived from the output base partition). Odd os
                    # groups land on partitions 64-127 of their own PSUM bank
                    # (separate banks, so start=True bank-clears don't collide).
                    H = P // 2
                    tb = tbs[0]
                    for kb in range(KB):
                        for os in range(NOS):
                            dst = (
                                ps[(0, os)][:H, :]
                                if os % 2 == 0
                                else ps[(0, os)][H:, :]
                            )
                            nc.tensor.matmul(
                                dst,
                                lhsT=lhs(pi, kb, tb, 0, 1)[:, :H],
                                rhs=w_slice(kb, os),
                                start=(kb == 0),
                                stop=(kb == KB - 1),
                            )
                    o_sb = op.tile([P, D], cdt, tag="o", name=f"oq_{pi}")
                    for os in range(NOS):
                        rows = slice(0, H) if os % 2 == 0 else slice(H, P)
                        nc.vector.tensor_copy(
                            o_sb[rows, os * NS : (os + 1) * NS],
                            ps[(0, os)][rows, :],
                        )
                        # Sync ring: idle at kernel end, while Scalar may
                        # still be draining the previous pair's 2 MB of output
                        nc.sync.dma_start(
                            out_d[tb * P : tb * P + H, os * NS : (os + 1) * NS],
                            o_sb[rows, os * NS : (os + 1) * NS],
                        )
                    continue
                # pair 0 follows the two-phase weight stream (os 0-1 while
                # the A halves land, then os 2-3); later pairs interleave
                # all four os per kb for 4-matmul LDWEIGHTS amortization.
                os_phases = [(0, 1), (2, 3)] if pi == 0 else [(0, 1, 2, 3)]
                for phase in os_phases:
                    for kb in range(KB):
                        for ti, tb in enumerate(tbs):
                            for os in phase:
                                nc.tensor.matmul(
                                    ps[(ti, os)][:],
                                    lhsT=lhs(pi, kb, tb, ti, len(tbs)),
                                    rhs=w_slice(kb, os),
                                    start=(kb == 0),
                                    stop=(kb == KB - 1),
                                )
                for ti, tb in enumerate(tbs):
                    o_sb = op.tile([P, D], cdt, tag="o", name=f"o_{pi}_{ti}")
                    for os in range(NOS):
                        nc.vector.tensor_copy(
                            o_sb[:, os * NS : (os + 1) * NS], ps[(ti, os)][:]
                        )
                        if last:
                            # tail: stream each 512-slice out as soon as its
                            # copy lands instead of one 1 MB DMA at the end
                            nc.scalar.dma_start(
                                out_d[tb * P : (tb + 1) * P, os * NS : (os + 1) * NS],
                                o_sb[:, os * NS : (os + 1) * NS],
                            )
                    if not last:
                        nc.scalar.dma_start(out_d[tb * P : (tb + 1) * P, :], o_sb[:])
    nc.compile()
    return nc


def _get_nc(C, compute_dt, last_m):
    key = (C, compute_dt, last_m)
    if key not in _cache:
        _cache[key] = _build(C, compute_dt, last_m)
    return _cache[key]


def kernel(tokens, weight, exp_ids, _trace=False, _compute_dt="float16"):
    _ensure_imports()
    from concourse.bass_utils import run_bass_kernel_spmd

    tokens = np.asarray(tokens)
    weight = np.asarray(weight)
    exp_ids = np.asarray(exp_ids)
    T = tokens.shape[0]

    order = np.argsort(exp_ids, kind="stable")
    counts = np.bincount(exp_ids, minlength=E)
    C = max(int(-(-counts.max() // P) * P), NS)

    starts = np.zeros(E + 1, dtype=np.int64)
    np.cumsum(counts, out=starts[1:])

    # Packed final block is valid when the last 128-block holds <= 64 real
    # tokens on every core and the block count is odd (lone final block).
    TB = C // P
    rest = int(counts.max()) - (TB - 1) * P
    last_m = 64 if (TB >= 3 and TB % 2 == 1 and rest <= 64) else 128

    npdt = _np_dt(_compute_dt)
    tokens_c = tokens.astype(npdt)
    weight_c = weight.astype(npdt)

    n0 = 2 * P if C // P >= 2 else P
    in_maps = []
    for e in range(E):
        idx = order[starts[e] : starts[e + 1]]
        xt = np.zeros((D, C), dtype=npdt)
        xt[:, : counts[e]] = tokens_c[idx].T
        # xt0: first-pair stationary blocks packed [p, kb*n0 + t] contiguously
        xt0 = np.ascontiguousarray(
            xt[:, :n0].reshape(KB, P, n0).transpose(1, 0, 2).reshape(P, KB * n0)
        )
        in_maps.append({"xt": xt, "xt0": xt0, "w": np.ascontiguousarray(weight_c[e])})

    nc = _get_nc(C, _compute_dt, last_m)
    res = run_bass_kernel_spmd(
        nc,
        in_maps,
        core_ids=list(range(E)),
        trace=_trace,
        trace_cores=list(range(E)) if _trace else None,
    )

    out = np.empty((T, D), dtype=np.float32)
    for e in range(E):
        idx = order[starts[e] : starts[e + 1]]
        out[idx] = res.results[e]["out"][: counts[e], :].astype(np.float32)
    if _trace:
        return out, res
    return out
